# revision 19
# baseline (speedup 1.0000x reference)
"""Trainium2 Bass kernel for AnsiToPixels (embedding_lookup, memory-bound).

Computation (per glyph cell):
  raw[y,x]  = sum_ch char[ch] * glyph[ch,y,x]          (256-ch dense "one-hot" matmul)
  fg[c]     = (0.5*fg_bold+0.5) * fg_color[c]
  bg[c]     = (0.5*bg_bold+0.5) * bg_color[c]
  out[y,x,c] = raw[y,x]*(fg[c]-bg[c]) + bg[c]

Sharding: pure data parallelism over batch B=128 -> 16 per core on 8 cores,
glyph table replicated. Each core processes 25600 cells.

v2 design (HBM-traffic-minimized):
  - Host pre-transposes the one-hot block to channel-major fp8e4m3
    charT[t][p=ch_lo][h*2560+cell] so the device needs NO PE transposes and
    reads 6.6MB instead of 26MB per core. Colors stay f32 (exact), cell-major.
  - Device: per 128-cell chunk, 2 accumulating fp8 matmuls (lhsT=charT chunk
    stationary, rhs=glyph half moving) -> raw[cell,pix] f32 in PSUM.
  - ACT copies each full PSUM bank (4 chunks, FD=512) to SBUF bf16; DVE then
    runs the fused blend out = raw*d + bg per (chunk,channel) as tensor_scalar
    with per-partition scalars — bf16 SBUF dense => 4x perf mode (~94ns/op).
  - Output written as one contiguous bf16 DMA per macro-tile in device layout
    [t][p][k][c][pix]; host reassembles to [B,320,640,3] f32 (cell = t*2560 +
    k*128 + p). Device HBM traffic ~27MB/core => ~75us DMA roofline.
"""

import os
import sys

import numpy as np

for _p in ("/opt/trn_rl_repo", "/root/.axon_site/_ro/trn_rl_repo"):
    if os.path.isdir(_p) and _p not in sys.path:
        sys.path.insert(0, _p)

import ml_dtypes  # noqa: E402

import concourse.bass as bass  # noqa: E402
import concourse.mybir as mybir  # noqa: E402
import concourse.tile as tile  # noqa: E402
from concourse import bacc  # noqa: E402
from concourse.bass_utils import run_bass_kernel_spmd  # noqa: E402


def _ensure_ntff_hook():
    """Register the axon NTFF profile hook if the image's antenv lacks it,
    so run_bass_kernel_spmd(trace=True) can capture HW exec time."""
    try:
        from antenv.axon_hooks import get_axon_ntff_profile_hook  # noqa: F401

        return
    except ImportError:
        pass
    try:
        import types

        import antenv
        from trn_agent_boot.trn_boot import _ntff_profile_via_ctypes

        hook = _ntff_profile_via_ctypes("/opt/axon/libaxon_pjrt.so")
        mod = types.ModuleType("antenv.axon_hooks")
        mod.get_axon_ntff_profile_hook = lambda: hook
        mod.set_axon_ntff_profile_hook = lambda h: None
        sys.modules["antenv.axon_hooks"] = mod
        antenv.axon_hooks = mod
    except Exception as e:  # profiling is best-effort
        print(f"NTFF hook registration failed: {e}", file=sys.stderr)


N_CORES = 8
B = 128
GRID_H, GRID_W = 20, 80
GLYPH_H, GLYPH_W = 16, 8
N_GLYPHS = 256
PIX = GLYPH_H * GLYPH_W  # 128

B_SHARD = B // N_CORES  # 16
CELLS = B_SHARD * GRID_H * GRID_W  # 25600
MT = 2560  # cells per macro-tile
NT = CELLS // MT  # 10 macro-tiles
KPT = MT // 128  # 128-cell chunks per macro-tile (20)
GROUPS = [(0, 8), (8, 8), (16, 4)]  # psum groups (start chunk, n chunks)

# blend engine assignment, weighted round-robin over (chunk, channel):
# measured marginal rates DVE ~240ns, ACT ~390ns (+copies), GpSimd ~550ns
# (GPS pays SBUF-port contention + sem overhead) -> 13 DVE / 5 ACT / 6 GPS
_D, _G, _A = 0, 1, 2
BLEND_PAT = [
    _D, _G, _D, _A, _D, _G, _D, _D, _A, _D, _A, _D,
    _G, _D, _D, _G, _D, _A, _D, _G, _D, _D, _A, _G,
]

F32 = mybir.dt.float32
BF16 = mybir.dt.bfloat16
F8 = mybir.dt.float8e4

NP_F8 = ml_dtypes.float8_e4m3
NP_BF16 = ml_dtypes.bfloat16


def _bcast_last(ap, n):
    """Append a stride-0 dim of size n to an AP (free-dim broadcast)."""
    return bass.AP(tensor=ap.tensor, offset=ap.offset, ap=[*ap.ap, [0, n]])


def build_kernel():
    nc = bacc.Bacc(
        "TRN2",
        target_bir_lowering=False,
        debug=False,
        enable_asserts=False,
        num_devices=N_CORES,
    )
    # charT[t][p=ch_lo][h*MT + cell] fp8, channel-major, host-pretransposed
    ct = nc.dram_tensor("ct", [NT, 128, 2 * MT], F8, kind="ExternalInput").ap()
    # colors[t][p][k][8] f32 where cell = t*MT + k*128 + p
    col = nc.dram_tensor("col", [NT, 128, KPT, 8], F32, kind="ExternalInput").ap()
    # glyph[p=ch_lo][h][pix] fp8
    gl = nc.dram_tensor("gl", [128, 2, PIX], F8, kind="ExternalInput").ap()
    # out[t][p][k][c][pix] bf16, cell = t*MT + k*128 + p
    outp = nc.dram_tensor(
        "out", [NT, 128, KPT, 3, PIX], BF16, kind="ExternalOutput"
    ).ap()

    with tile.TileContext(nc) as tc:
        with (
            tc.tile_pool(name="const", bufs=1) as const,
            tc.tile_pool(name="char", bufs=4) as char_pool,
            tc.tile_pool(name="colp", bufs=4) as col_pool,
            tc.tile_pool(name="raw", bufs=8) as raw_pool,
            tc.tile_pool(name="outsb", bufs=4) as out_pool,
            tc.tile_pool(name="grp", bufs=3) as grp_pool,
            tc.tile_pool(name="psR", bufs=4, space="PSUM") as psR,
        ):
            gsb = const.tile([128, 2, PIX], F8)
            nc.sync.dma_start(out=gsb[:, :, :], in_=gl[:, :, :])

            for t in range(NT):
                ctt = char_pool.tile([128, 2 * MT], F8)
                nc.sync.dma_start(out=ctt[:, :], in_=ct[t, :, :])
                colt = col_pool.tile([128, KPT, 8], F32)
                nc.sync.dma_start(out=colt[:, :, :], in_=col[t, :, :, :])

                # d = fg-bg and bg, per cell (partition p, chunk k), f32
                # (ISA requires f32 scalar APs for mult); on DVE — cheaper
                # there than on GpSimd, which is blend-rate-limited
                sf = grp_pool.tile([128, KPT], F32, tag="sf")
                sb = grp_pool.tile([128, KPT], F32, tag="sb")
                fg = grp_pool.tile([128, KPT, 3], F32, tag="fg")
                bg = grp_pool.tile([128, KPT, 3], F32, tag="bg")
                d = grp_pool.tile([128, KPT, 3], F32, tag="d")
                nc.vector.tensor_scalar(
                    out=sf[:, :],
                    in0=colt[:, :, 0],
                    scalar1=0.5,
                    scalar2=0.5,
                    op0=mybir.AluOpType.mult,
                    op1=mybir.AluOpType.add,
                )
                nc.vector.tensor_scalar(
                    out=sb[:, :],
                    in0=colt[:, :, 4],
                    scalar1=0.5,
                    scalar2=0.5,
                    op0=mybir.AluOpType.mult,
                    op1=mybir.AluOpType.add,
                )
                nc.vector.tensor_mul(
                    fg[:, :, :], colt[:, :, 1:4], _bcast_last(sf[:, :], 3)
                )
                nc.vector.tensor_mul(
                    bg[:, :, :], colt[:, :, 5:8], _bcast_last(sb[:, :], 3)
                )
                nc.vector.tensor_sub(d[:, :, :], fg[:, :, :], bg[:, :, :])

                out_sb = out_pool.tile([128, KPT, 3, PIX], BF16)
                for k0, gn in GROUPS:
                    ps = psR.tile([128, 8 * PIX], F32)
                    for j in range(gn):
                        k = k0 + j
                        nc.tensor.matmul(
                            ps[:, j * 128 : (j + 1) * 128],
                            ctt[:, k * 128 : (k + 1) * 128],
                            gsb[:, 0, :],
                            start=True,
                            stop=False,
                        )
                        nc.tensor.matmul(
                            ps[:, j * 128 : (j + 1) * 128],
                            ctt[:, MT + k * 128 : MT + (k + 1) * 128],
                            gsb[:, 1, :],
                            start=False,
                            stop=True,
                        )
                    # one wide ACT copy+cast of the psum group -> bf16 SBUF
                    raws = raw_pool.tile([128, 8, PIX], BF16)
                    nc.scalar.copy(
                        raws[:, 0:gn, :],
                        ps[:, 0 : gn * PIX].rearrange("p (j x) -> p j x", x=PIX),
                    )
                    # fused blends spread over DVE / GpSimd / ACT
                    for j in range(gn):
                        k = k0 + j
                        for c in range(3):
                            eng = BLEND_PAT[(k * 3 + c) % len(BLEND_PAT)]
                            if eng == _A:
                                nc.scalar.activation(
                                    out_sb[:, k, c, :],
                                    raws[:, j, :],
                                    mybir.ActivationFunctionType.Identity,
                                    bias=bg[:, k, c : c + 1],
                                    scale=d[:, k, c : c + 1],
                                )
                            else:
                                e = nc.vector if eng == _D else nc.gpsimd
                                e.tensor_scalar(
                                    out=out_sb[:, k, c, :],
                                    in0=raws[:, j, :],
                                    scalar1=d[:, k, c : c + 1],
                                    scalar2=bg[:, k, c : c + 1],
                                    op0=mybir.AluOpType.mult,
                                    op1=mybir.AluOpType.add,
                                )

                nc.scalar.dma_start(out=outp[t, :, :, :, :], in_=out_sb[:, :, :, :])

    nc.compile()
    return nc


_NC = None


def _get_nc():
    global _NC
    if _NC is None:
        _NC = build_kernel()
    return _NC


def _prep_core(flat):
    """flat: [CELLS, 264] f32 for one core -> device input dict."""
    oh = np.ascontiguousarray(flat[:, :256].T)  # [256, CELLS]
    # [2, 128, NT, MT] -> [t, p, h, cell] -> [NT, 128, 2*MT] fp8
    ctv = oh.reshape(2, 128, NT, MT).transpose(2, 1, 0, 3)
    ct8 = np.ascontiguousarray(ctv.astype(NP_F8).reshape(NT, 128, 2 * MT))
    # colors: cell = t*MT + k*128 + p -> [NT, 128, KPT, 8] f32
    colv = flat[:, 256:264].reshape(NT, KPT, 128, 8).transpose(0, 2, 1, 3)
    colc = np.ascontiguousarray(colv)
    return {"ct": ct8, "col": colc}


def run(data, char_matrix, trace=False, want_res=False):
    data = np.asarray(data, dtype=np.float32)
    assert data.shape == (B, GRID_H, GRID_W, 264), data.shape
    g = np.asarray(char_matrix, dtype=np.float32).reshape(N_GLYPHS, PIX)
    gl8 = np.ascontiguousarray(
        g.reshape(2, 128, PIX).transpose(1, 0, 2).astype(NP_F8)
    )

    in_maps = []
    for i in range(N_CORES):
        flat = data[i * B_SHARD : (i + 1) * B_SHARD].reshape(CELLS, 264)
        m = _prep_core(flat)
        m["gl"] = gl8
        in_maps.append(m)

    nc = _get_nc()
    if trace:
        _ensure_ntff_hook()
    res = run_bass_kernel_spmd(
        nc, in_maps, core_ids=list(range(N_CORES)), trace=trace
    )

    outs = []
    for r in res.results:
        x = np.asarray(r["out"]).reshape(NT, 128, KPT, 3, GLYPH_H, GLYPH_W)
        # [t,p,k,c,gy,gx] -> cell-major [t,k,p,...] -> [b,h,w,c,gy,gx]
        x = x.transpose(0, 2, 1, 3, 4, 5).reshape(
            B_SHARD, GRID_H, GRID_W, 3, GLYPH_H, GLYPH_W
        )
        # -> [b, h, gy, w, gx, c] f32 -> [b, 320, 640, 3]
        x = x.transpose(0, 1, 4, 2, 5, 3).astype(np.float32)
        outs.append(x.reshape(B_SHARD, GRID_H * GLYPH_H, GRID_W * GLYPH_W, 3))
    out = np.concatenate(outs, axis=0)
    if want_res:
        return out, res.exec_time_ns, res
    return out, res.exec_time_ns


def kernel(data, char_matrix):
    out, _ = run(data, char_matrix, trace=False)
    return out


# revision 22
# speedup vs baseline: 1.0512x; 1.0512x over previous
"""Trainium2 Bass kernel for AnsiToPixels (embedding_lookup, memory-bound).

Computation (per glyph cell):
  raw[y,x]  = sum_ch char[ch] * glyph[ch,y,x]          (256-ch dense "one-hot" matmul)
  fg[c]     = (0.5*fg_bold+0.5) * fg_color[c]
  bg[c]     = (0.5*bg_bold+0.5) * bg_color[c]
  out[y,x,c] = raw[y,x]*(fg[c]-bg[c]) + bg[c]

Sharding: pure data parallelism over batch B=128 -> 16 per core on 8 cores,
glyph table replicated. Each core processes 25600 cells.

v2 design (HBM-traffic-minimized):
  - Host pre-transposes the one-hot block to channel-major fp8e4m3
    charT[t][p=ch_lo][h*2560+cell] so the device needs NO PE transposes and
    reads 6.6MB instead of 26MB per core. Colors stay f32 (exact), cell-major.
  - Device: per 128-cell chunk, 2 accumulating fp8 matmuls (lhsT=charT chunk
    stationary, rhs=glyph half moving) -> raw[cell,pix] f32 in PSUM.
  - ACT copies each full PSUM bank (4 chunks, FD=512) to SBUF bf16; DVE then
    runs the fused blend out = raw*d + bg per (chunk,channel) as tensor_scalar
    with per-partition scalars — bf16 SBUF dense => 4x perf mode (~94ns/op).
  - Output written as one contiguous bf16 DMA per macro-tile in device layout
    [t][p][k][c][pix]; host reassembles to [B,320,640,3] f32 (cell = t*2560 +
    k*128 + p). Device HBM traffic ~27MB/core => ~75us DMA roofline.
"""

import os
import sys

import numpy as np

for _p in ("/opt/trn_rl_repo", "/root/.axon_site/_ro/trn_rl_repo"):
    if os.path.isdir(_p) and _p not in sys.path:
        sys.path.insert(0, _p)

import ml_dtypes  # noqa: E402

import concourse.bass as bass  # noqa: E402
import concourse.mybir as mybir  # noqa: E402
import concourse.tile as tile  # noqa: E402
from concourse import bacc  # noqa: E402
from concourse.bass_utils import run_bass_kernel_spmd  # noqa: E402


def _ensure_ntff_hook():
    """Register the axon NTFF profile hook if the image's antenv lacks it,
    so run_bass_kernel_spmd(trace=True) can capture HW exec time."""
    try:
        from antenv.axon_hooks import get_axon_ntff_profile_hook  # noqa: F401

        return
    except ImportError:
        pass
    try:
        import types

        import antenv
        from trn_agent_boot.trn_boot import _ntff_profile_via_ctypes

        hook = _ntff_profile_via_ctypes("/opt/axon/libaxon_pjrt.so")
        mod = types.ModuleType("antenv.axon_hooks")
        mod.get_axon_ntff_profile_hook = lambda: hook
        mod.set_axon_ntff_profile_hook = lambda h: None
        sys.modules["antenv.axon_hooks"] = mod
        antenv.axon_hooks = mod
    except Exception as e:  # profiling is best-effort
        print(f"NTFF hook registration failed: {e}", file=sys.stderr)


N_CORES = 8
B = 128
GRID_H, GRID_W = 20, 80
GLYPH_H, GLYPH_W = 16, 8
N_GLYPHS = 256
PIX = GLYPH_H * GLYPH_W  # 128

B_SHARD = B // N_CORES  # 16
CELLS = B_SHARD * GRID_H * GRID_W  # 25600
MT = 2560  # cells per macro-tile
NT = CELLS // MT  # 10 macro-tiles
KPT = MT // 128  # 128-cell chunks per macro-tile (20)
GROUPS = [(0, 8), (8, 8), (16, 4)]  # psum groups (start chunk, n chunks)

# blend engine assignment: contiguous runs per psum group (one sem wait per
# run instead of per op — GpSimd sem ops cost ~350ns each).
# measured marginal rates DVE ~240ns, ACT ~390ns (+copies), GpSimd ~550ns.
# per 8-chunk group (24 ops): GPS 6, ACT 5, DVE 13; per 4-chunk: 3/2/7
_D, _G, _A = 0, 1, 2
GROUP_SPLIT = {8: (6, 5), 4: (3, 2)}  # (n_gps, n_act); rest DVE

F32 = mybir.dt.float32
BF16 = mybir.dt.bfloat16
F8 = mybir.dt.float8e4

NP_F8 = ml_dtypes.float8_e4m3
NP_BF16 = ml_dtypes.bfloat16


def _bcast_last(ap, n):
    """Append a stride-0 dim of size n to an AP (free-dim broadcast)."""
    return bass.AP(tensor=ap.tensor, offset=ap.offset, ap=[*ap.ap, [0, n]])


def build_kernel():
    nc = bacc.Bacc(
        "TRN2",
        target_bir_lowering=False,
        debug=False,
        enable_asserts=False,
        num_devices=N_CORES,
    )
    # charT[t][p=ch_lo][h*MT + cell] fp8, channel-major, host-pretransposed
    ct = nc.dram_tensor("ct", [NT, 128, 2 * MT], F8, kind="ExternalInput").ap()
    # colors[t][p][k][8] f32 where cell = t*MT + k*128 + p
    col = nc.dram_tensor("col", [NT, 128, KPT, 8], F32, kind="ExternalInput").ap()
    # glyph[p=ch_lo][h][pix] fp8
    gl = nc.dram_tensor("gl", [128, 2, PIX], F8, kind="ExternalInput").ap()
    # out[t][p][k][c][pix] bf16, cell = t*MT + k*128 + p
    outp = nc.dram_tensor(
        "out", [NT, 128, KPT, 3, PIX], BF16, kind="ExternalOutput"
    ).ap()

    with tile.TileContext(nc) as tc:
        with (
            tc.tile_pool(name="const", bufs=1) as const,
            tc.tile_pool(name="char", bufs=4) as char_pool,
            tc.tile_pool(name="colp", bufs=4) as col_pool,
            tc.tile_pool(name="raw", bufs=8) as raw_pool,
            tc.tile_pool(name="outsb", bufs=4) as out_pool,
            tc.tile_pool(name="grp", bufs=3) as grp_pool,
            tc.tile_pool(name="psR", bufs=4, space="PSUM") as psR,
        ):
            gsb = const.tile([128, 2, PIX], F8)
            nc.sync.dma_start(out=gsb[:, :, :], in_=gl[:, :, :])

            for t in range(NT):
                ctt = char_pool.tile([128, 2 * MT], F8)
                nc.sync.dma_start(out=ctt[:, :], in_=ct[t, :, :])
                colt = col_pool.tile([128, KPT, 8], F32)
                nc.sync.dma_start(out=colt[:, :, :], in_=col[t, :, :, :])

                # d = fg-bg and bg, per cell (partition p, chunk k), f32
                # (ISA requires f32 scalar APs for mult); on DVE — cheaper
                # there than on GpSimd, which is blend-rate-limited
                sf = grp_pool.tile([128, KPT], F32, tag="sf")
                sb = grp_pool.tile([128, KPT], F32, tag="sb")
                fg = grp_pool.tile([128, KPT, 3], F32, tag="fg")
                bg = grp_pool.tile([128, KPT, 3], F32, tag="bg")
                d = grp_pool.tile([128, KPT, 3], F32, tag="d")
                nc.vector.tensor_scalar(
                    out=sf[:, :],
                    in0=colt[:, :, 0],
                    scalar1=0.5,
                    scalar2=0.5,
                    op0=mybir.AluOpType.mult,
                    op1=mybir.AluOpType.add,
                )
                nc.vector.tensor_scalar(
                    out=sb[:, :],
                    in0=colt[:, :, 4],
                    scalar1=0.5,
                    scalar2=0.5,
                    op0=mybir.AluOpType.mult,
                    op1=mybir.AluOpType.add,
                )
                nc.vector.tensor_mul(
                    fg[:, :, :], colt[:, :, 1:4], _bcast_last(sf[:, :], 3)
                )
                nc.vector.tensor_mul(
                    bg[:, :, :], colt[:, :, 5:8], _bcast_last(sb[:, :], 3)
                )
                nc.vector.tensor_sub(d[:, :, :], fg[:, :, :], bg[:, :, :])

                out_sb = out_pool.tile([128, KPT, 3, PIX], BF16)
                for k0, gn in GROUPS:
                    ps = psR.tile([128, 8 * PIX], F32)
                    for j in range(gn):
                        k = k0 + j
                        nc.tensor.matmul(
                            ps[:, j * 128 : (j + 1) * 128],
                            ctt[:, k * 128 : (k + 1) * 128],
                            gsb[:, 0, :],
                            start=True,
                            stop=False,
                        )
                        nc.tensor.matmul(
                            ps[:, j * 128 : (j + 1) * 128],
                            ctt[:, MT + k * 128 : MT + (k + 1) * 128],
                            gsb[:, 1, :],
                            start=False,
                            stop=True,
                        )
                    # one wide ACT copy+cast of the psum group -> bf16 SBUF
                    raws = raw_pool.tile([128, 8, PIX], BF16)
                    nc.scalar.copy(
                        raws[:, 0:gn, :],
                        ps[:, 0 : gn * PIX].rearrange("p (j x) -> p j x", x=PIX),
                    )
                    # fused blends spread over DVE / GpSimd / ACT in
                    # contiguous runs within the group
                    n_gps, n_act = GROUP_SPLIT[gn]
                    for j in range(gn):
                        k = k0 + j
                        for c in range(3):
                            i = j * 3 + c
                            if i < n_gps:
                                eng = _G
                            elif i < n_gps + n_act:
                                eng = _A
                            else:
                                eng = _D
                            if eng == _A:
                                nc.scalar.activation(
                                    out_sb[:, k, c, :],
                                    raws[:, j, :],
                                    mybir.ActivationFunctionType.Identity,
                                    bias=bg[:, k, c : c + 1],
                                    scale=d[:, k, c : c + 1],
                                )
                            else:
                                e = nc.vector if eng == _D else nc.gpsimd
                                e.tensor_scalar(
                                    out=out_sb[:, k, c, :],
                                    in0=raws[:, j, :],
                                    scalar1=d[:, k, c : c + 1],
                                    scalar2=bg[:, k, c : c + 1],
                                    op0=mybir.AluOpType.mult,
                                    op1=mybir.AluOpType.add,
                                )

                if t == NT - 1:
                    # split the final store so the kernel tail is ~3us not ~6
                    nc.scalar.dma_start(
                        out=outp[t, :, 0:10, :, :], in_=out_sb[:, 0:10, :, :]
                    )
                    nc.scalar.dma_start(
                        out=outp[t, :, 10:KPT, :, :], in_=out_sb[:, 10:KPT, :, :]
                    )
                else:
                    nc.scalar.dma_start(
                        out=outp[t, :, :, :, :], in_=out_sb[:, :, :, :]
                    )

    nc.compile()
    return nc


_NC = None


def _get_nc():
    global _NC
    if _NC is None:
        _NC = build_kernel()
    return _NC


def _prep_core(flat):
    """flat: [CELLS, 264] f32 for one core -> device input dict."""
    oh = np.ascontiguousarray(flat[:, :256].T)  # [256, CELLS]
    # [2, 128, NT, MT] -> [t, p, h, cell] -> [NT, 128, 2*MT] fp8
    ctv = oh.reshape(2, 128, NT, MT).transpose(2, 1, 0, 3)
    ct8 = np.ascontiguousarray(ctv.astype(NP_F8).reshape(NT, 128, 2 * MT))
    # colors: cell = t*MT + k*128 + p -> [NT, 128, KPT, 8] f32
    colv = flat[:, 256:264].reshape(NT, KPT, 128, 8).transpose(0, 2, 1, 3)
    colc = np.ascontiguousarray(colv)
    return {"ct": ct8, "col": colc}


def run(data, char_matrix, trace=False, want_res=False):
    data = np.asarray(data, dtype=np.float32)
    assert data.shape == (B, GRID_H, GRID_W, 264), data.shape
    g = np.asarray(char_matrix, dtype=np.float32).reshape(N_GLYPHS, PIX)
    gl8 = np.ascontiguousarray(
        g.reshape(2, 128, PIX).transpose(1, 0, 2).astype(NP_F8)
    )

    in_maps = []
    for i in range(N_CORES):
        flat = data[i * B_SHARD : (i + 1) * B_SHARD].reshape(CELLS, 264)
        m = _prep_core(flat)
        m["gl"] = gl8
        in_maps.append(m)

    nc = _get_nc()
    if trace:
        _ensure_ntff_hook()
    res = run_bass_kernel_spmd(
        nc, in_maps, core_ids=list(range(N_CORES)), trace=trace
    )

    outs = []
    for r in res.results:
        x = np.asarray(r["out"]).reshape(NT, 128, KPT, 3, GLYPH_H, GLYPH_W)
        # [t,p,k,c,gy,gx] -> cell-major [t,k,p,...] -> [b,h,w,c,gy,gx]
        x = x.transpose(0, 2, 1, 3, 4, 5).reshape(
            B_SHARD, GRID_H, GRID_W, 3, GLYPH_H, GLYPH_W
        )
        # -> [b, h, gy, w, gx, c] f32 -> [b, 320, 640, 3]
        x = x.transpose(0, 1, 4, 2, 5, 3).astype(np.float32)
        outs.append(x.reshape(B_SHARD, GRID_H * GLYPH_H, GRID_W * GLYPH_W, 3))
    out = np.concatenate(outs, axis=0)
    if want_res:
        return out, res.exec_time_ns, res
    return out, res.exec_time_ns


def kernel(data, char_matrix):
    out, _ = run(data, char_matrix, trace=False)
    return out
